# revision 1
# baseline (speedup 1.0000x reference)
"""Trainium2 Bass kernel for nn_De_conv_batched_multimasks (segment_reduce).

Self-contained: accepts FULL inputs, shards the B*N supervoxel areas across
8 NeuronCores (fully data-parallel), runs one SPMD Bass/Tile kernel, and
gathers the full [B, N] output.

Exact (fp32) reformulations of the reference used here:
 - or_simple(x, y) = 2x - x^2 (y-independent), so
   eroded = (1 - edge_diff*mc)^2 * mc with edge_diff = a_b + a_f - 2*a_b*a_f
   (only row shifts survive; the column-shift term cancels).
 - Every sin argument in the diff_round chains provably lies in [0,1] or
   [-1,0]; sin(2*pi*x) is evaluated as -sin(2*pi*x -+ pi) with the +-pi in
   the ACT bias (free), keeping the Sin spline inside its valid range
   (|arg| <= pi; the hardware spline diverges past ~3.8).
 - Additive integer offsets ride on sin periodicity (chains track t-1).

Engine split per chunk (128 partitions x 8 areas each):
  ACT : 6 Sins + 2 Squares + d-shift (Identity, bias=1) + 2 tiny mid Sins
  DVE : diff_round combine STTs, r-combine, h, y0, reductions, finals
  POOL: half the affine TTs, erosion TTs, z/y1, q01/q23/mc, border memsets

The builder is exec'd from a stable synthetic filename so the emitted BIR
(which embeds per-instruction source-location debug info) is byte-identical
regardless of the directory this file lives in — the neuron compile cache
then hits across directories.
"""

import numpy as np

import concourse.bacc as bacc
import concourse.mybir as mybir
from concourse.tile import TileContext

B, N, W, H = 8, 8192, 8, 8
PX = W * H
CH = 4
NCORES = 8
A_TOT = B * N
A_CORE = A_TOT // NCORES
P = 128

_BUILDER_SRC = r'''
F32 = mybir.dt.float32
AX = mybir.AxisListType
OP = mybir.AluOpType
ACTF = mybir.ActivationFunctionType

PI = float(np.pi)
TWO_PI = 2.0 * PI
INV_2PI = 1.0 / TWO_PI
EPS = 1e-8


def _build(g=8, bufs_small=3, bufs_big=3, aff_pool=2):
    G = g
    CHUNKS = A_CORE // (P * G)
    nc = bacc.Bacc("TRN2", target_bir_lowering=False, debug=False,
                   num_devices=NCORES)
    img_d = nc.dram_tensor("img", [A_CORE, PX], F32, kind="ExternalInput")
    mask_d = nc.dram_tensor("mask", [A_CORE, PX * CH], F32, kind="ExternalInput")
    mid_d = nc.dram_tensor("mid", [A_CORE, CH], F32, kind="ExternalInput")
    edge_d = nc.dram_tensor("edge", [A_CORE, PX], F32, kind="ExternalInput")
    out_d = nc.dram_tensor("out", [A_CORE], F32, kind="ExternalOutput")

    img_v = img_d.ap().rearrange("(c p g) x -> c p (g x)", p=P, g=G)
    mask_v = mask_d.ap().rearrange("(c p g) x -> c p (g x)", p=P, g=G)
    mid_v = mid_d.ap().rearrange("(c p g) x -> c p (g x)", p=P, g=G)
    edge_v = edge_d.ap().rearrange("(c p g) x -> c p (g x)", p=P, g=G)
    out_v = out_d.ap().rearrange("(c p g) -> c p g", p=P, g=G)

    FD = G * PX * CH
    FE = G * PX

    with TileContext(nc) as tc:
        with tc.tile_pool(name="cpool", bufs=1) as cpool, \
             tc.tile_pool(name="pool", bufs=bufs_big) as pool:
            bias_n = cpool.tile([P, 1], F32, name="bias_n")
            bias_p = cpool.tile([P, 1], F32, name="bias_p")
            nc.vector.memset(bias_n[:, :], -PI)
            nc.vector.memset(bias_p[:, :], PI)
            BNp = bias_n[:, :]
            BPp = bias_p[:, :]

            for c in range(CHUNKS):
                mask_t = pool.tile([P, FD], F32, name="mask_t")
                img_t = pool.tile([P, FE], F32, name="img_t")
                edge_t = pool.tile([P, FE], F32, name="edge_t")
                mid_t = pool.tile([P, G * CH], F32, name="mid_t")
                nc.sync.dma_start(mask_t[:, :], mask_v[c])
                nc.sync.dma_start(img_t[:, :], img_v[c])
                nc.sync.dma_start(edge_t[:, :], edge_v[c])
                nc.sync.dma_start(mid_t[:, :], mid_v[c])

                # compact per-area chain: b2 = hdr(mid), b2p = 2*b2 - 1
                sm = pool.tile([P, G * CH], F32, name="sm")
                m1t = pool.tile([P, G * CH], F32, name="m1t")
                b2 = pool.tile([P, G * CH], F32, name="b2")
                b2p = pool.tile([P, G * CH], F32, name="b2p")
                nc.scalar.activation(sm[:, :], mid_t[:, :], ACTF.Sin,
                                     scale=TWO_PI, bias=BNp)
                nc.vector.scalar_tensor_tensor(m1t[:, :], sm[:, :], INV_2PI,
                                               mid_t[:, :], op0=OP.mult,
                                               op1=OP.add)
                nc.scalar.activation(sm[:, :], m1t[:, :], ACTF.Sin,
                                     scale=TWO_PI, bias=BNp)
                nc.vector.scalar_tensor_tensor(b2[:, :], sm[:, :], INV_2PI,
                                               m1t[:, :], op0=OP.mult,
                                               op1=OP.add)
                nc.vector.tensor_scalar(b2p[:, :], b2[:, :], 2.0, -1.0,
                                        op0=OP.mult, op1=OP.add)
                b2_g = b2[:, :].rearrange("p (g c) -> p g c", g=G)
                b2p_g = b2p[:, :].rearrange("p (g c) -> p g c", g=G)

                # mask chain over [P, G*PX*CH], 3 rotating tiles
                sA = pool.tile([P, FD], F32, name="sA")
                cA = pool.tile([P, FD], F32, name="cA")

                def v4(t):
                    return t[:, :].rearrange("p (g x c) -> p g x c", g=G, c=CH)

                nc.scalar.activation(sA[:, :], mask_t[:, :], ACTF.Sin,
                                     scale=TWO_PI, bias=BNp)            # s1
                nc.vector.scalar_tensor_tensor(cA[:, :], sA[:, :], INV_2PI,
                                               mask_t[:, :], op0=OP.mult,
                                               op1=OP.add)              # a1
                nc.scalar.activation(mask_t[:, :], cA[:, :], ACTF.Sin,
                                     scale=TWO_PI, bias=BNp)            # s2
                nc.vector.scalar_tensor_tensor(sA[:, :], mask_t[:, :], INV_2PI,
                                               cA[:, :], op0=OP.mult,
                                               op1=OP.add)              # a2
                # affine: ta = a2*b2p -> cA ; tb = t-1 = ta - b2 -> mask_t
                for ci in range(CH):
                    eng = nc.gpsimd if ci < aff_pool else nc.vector
                    bc1 = b2p_g[:, :, ci].unsqueeze(2).broadcast_to([P, G, PX])
                    eng.tensor_tensor(v4(cA)[:, :, :, ci],
                                      v4(sA)[:, :, :, ci], bc1, op=OP.mult)
                for ci in range(CH):
                    eng = nc.gpsimd if ci < aff_pool else nc.vector
                    bc0 = b2_g[:, :, ci].unsqueeze(2).broadcast_to([P, G, PX])
                    eng.tensor_tensor(v4(mask_t)[:, :, :, ci],
                                      v4(cA)[:, :, :, ci], bc0, op=OP.subtract)
                nc.scalar.activation(sA[:, :], mask_t[:, :], ACTF.Sin,
                                     scale=TWO_PI, bias=BPp)            # s3
                nc.vector.scalar_tensor_tensor(cA[:, :], sA[:, :], INV_2PI,
                                               mask_t[:, :], op0=OP.mult,
                                               op1=OP.add)              # t1
                nc.scalar.activation(sA[:, :], cA[:, :], ACTF.Sin,
                                     scale=TWO_PI, bias=BPp)            # s4
                nc.vector.scalar_tensor_tensor(mask_t[:, :], sA[:, :], INV_2PI,
                                               cA[:, :], op0=OP.mult,
                                               op1=OP.add)              # cb
                nc.scalar.activation(sA[:, :], mask_t[:, :], ACTF.Sin,
                                     scale=TWO_PI, bias=BPp)            # s5
                nc.vector.scalar_tensor_tensor(cA[:, :], sA[:, :], INV_2PI,
                                               mask_t[:, :], op0=OP.mult,
                                               op1=OP.add)              # db
                nc.scalar.activation(mask_t[:, :], cA[:, :], ACTF.Identity,
                                     bias=1.0)                          # d

                # and-tree
                q = pool.tile([P, G * PX * 2], F32, name="q")
                sq = pool.tile([P, G * PX * 2], F32, name="sq",
                               bufs=bufs_small)
                r = pool.tile([P, G * PX * 2], F32, name="r",
                              bufs=bufs_small)
                d_v = v4(mask_t)
                q_v = q[:, :].rearrange("p (g x u) -> p g x u", g=G, u=2)
                nc.vector.tensor_tensor(q_v[:, :, :, 0], d_v[:, :, :, 0],
                                        d_v[:, :, :, 1], op=OP.mult)
                nc.vector.tensor_tensor(q_v[:, :, :, 1], d_v[:, :, :, 2],
                                        d_v[:, :, :, 3], op=OP.mult)
                nc.scalar.activation(sq[:, :], q[:, :], ACTF.Sin,
                                     scale=TWO_PI, bias=BNp)
                nc.vector.scalar_tensor_tensor(r[:, :], sq[:, :], INV_2PI,
                                               q[:, :], op0=OP.mult,
                                               op1=OP.add)
                r_v = r[:, :].rearrange("p (g x u) -> p g x u", g=G, u=2)

                # mc in padded [P, G, 10, 8] tile (rows 0 and 9 zeroed)
                mcp = pool.tile([P, G * 80], F32, name="mcp")
                mcp_v = mcp[:, :].rearrange("p (g w h) -> p g w h", g=G, w=10)
                mcp3 = mcp[:, :].rearrange("p (g e) -> p g e", g=G)
                nc.gpsimd.memset(mcp_v[:, :, 0, :], 0.0)
                nc.gpsimd.memset(mcp_v[:, :, 9, :], 0.0)
                mcc = mcp3[:, :, 8:72]
                nc.gpsimd.tensor_tensor(mcc, r_v[:, :, :, 0], r_v[:, :, :, 1],
                                        op=OP.mult)

                # erosion: sum(eroded * edge)
                ab = mcp3[:, :, 16:80]
                af = mcp3[:, :, 0:64]
                e1 = pool.tile([P, FE], F32, name="e1", bufs=bufs_small)
                e2 = pool.tile([P, FE], F32, name="e2", bufs=bufs_small)
                hh = pool.tile([P, FE], F32, name="hh", bufs=bufs_small)

                def vE(t):
                    return t[:, :].rearrange("p (g x) -> p g x", g=G)

                nc.gpsimd.tensor_tensor(vE(e1), ab, af, op=OP.mult)
                nc.gpsimd.tensor_tensor(vE(e2), ab, af, op=OP.add)
                nc.vector.scalar_tensor_tensor(hh[:, :], e1[:, :], -2.0,
                                               e2[:, :], op0=OP.mult,
                                               op1=OP.add)
                nc.gpsimd.tensor_tensor(vE(e2), vE(hh), mcc, op=OP.mult)
                nc.scalar.activation(e1[:, :], e2[:, :], ACTF.Square,
                                     scale=-1.0, bias=1.0)
                nc.gpsimd.tensor_tensor(vE(hh), mcc, vE(edge_t), op=OP.mult)
                nc.gpsimd.tensor_tensor(e2[:, :], e1[:, :], hh[:, :],
                                        op=OP.mult)
                sw2 = pool.tile([P, G], F32, name="sw2")
                nc.vector.reduce_sum(sw2[:, :], vE(e2), axis=AX.X)

                # stats
                z = pool.tile([P, FE], F32, name="z", bufs=bufs_small)
                nc.gpsimd.tensor_tensor(vE(z), mcc, vE(img_t), op=OP.mult)
                sz = pool.tile([P, G], F32, name="sz")
                smc = pool.tile([P, G], F32, name="smc")
                nc.vector.reduce_sum(sz[:, :], vE(z), axis=AX.X)
                nc.vector.reduce_sum(smc[:, :], mcc, axis=AX.X)
                rec = pool.tile([P, G], F32, name="rec")
                nc.vector.tensor_scalar(rec[:, :], smc[:, :], EPS, None,
                                        op0=OP.add)
                nc.vector.reciprocal(rec[:, :], rec[:, :])
                meann = pool.tile([P, G], F32, name="meann")
                nc.vector.tensor_tensor(meann[:, :], sz[:, :], rec[:, :],
                                        op=OP.mult)
                meann_bc = meann[:, :].unsqueeze(2).broadcast_to([P, G, PX])
                nc.vector.scalar_tensor_tensor(vE(hh), meann_bc, -1.0, vE(z),
                                               op0=OP.mult, op1=OP.add)   # y0
                nc.gpsimd.tensor_tensor(vE(e1), vE(hh), mcc, op=OP.mult)  # y1
                nc.scalar.activation(e2[:, :], e1[:, :], ACTF.Square)     # y2
                sy2 = pool.tile([P, G], F32, name="sy2")
                nc.vector.reduce_sum(sy2[:, :], vE(e2), axis=AX.X)
                varr = pool.tile([P, G], F32, name="varr")
                nc.vector.tensor_tensor(varr[:, :], sy2[:, :], rec[:, :],
                                        op=OP.mult)
                outv = pool.tile([P, G], F32, name="outv")
                nc.vector.scalar_tensor_tensor(outv[:, :], varr[:, :],
                                               1000.0 / PX, sw2[:, :],
                                               op0=OP.mult, op1=OP.mult)
                nc.sync.dma_start(out_v[c], outv[:, :])
    nc.compile()
    return nc
'''

_ns = {
    "np": np,
    "bacc": bacc,
    "mybir": mybir,
    "TileContext": TileContext,
    "B": B, "N": N, "W": W, "H": H, "PX": PX, "CH": CH,
    "NCORES": NCORES, "A_TOT": A_TOT, "A_CORE": A_CORE, "P": P,
}
exec(compile(_BUILDER_SRC, "<bass_kernel>", "exec"), _ns)
_build = _ns["_build"]

_NC = None


def _get_nc():
    global _NC
    if _NC is None:
        _NC = _build()
    return _NC


def kernel(resized_image, mask_combined, initial_masks, edge_map, mask_index=0):
    from concourse.bass_utils import run_bass_kernel_spmd
    img = np.ascontiguousarray(resized_image, dtype=np.float32).reshape(A_TOT, PX)
    mask = np.ascontiguousarray(mask_combined, dtype=np.float32).reshape(A_TOT, PX * CH)
    mid = np.ascontiguousarray(initial_masks, dtype=np.float32).reshape(A_TOT, CH)
    edge = np.ascontiguousarray(edge_map, dtype=np.float32).reshape(A_TOT, PX)

    nc = _get_nc()
    in_maps = []
    for k in range(NCORES):
        sl = slice(k * A_CORE, (k + 1) * A_CORE)
        in_maps.append({
            "img": np.ascontiguousarray(img[sl]),
            "mask": np.ascontiguousarray(mask[sl]),
            "mid": np.ascontiguousarray(mid[sl]),
            "edge": np.ascontiguousarray(edge[sl]),
        })
    res = run_bass_kernel_spmd(nc, in_maps, core_ids=list(range(NCORES)))
    out = np.concatenate([res.results[k]["out"].reshape(-1)
                          for k in range(NCORES)])
    return out.reshape(B, N).astype(np.float32)



# revision 18
# speedup vs baseline: 2.1211x; 2.1211x over previous
"""Trainium2 Bass kernel for nn_De_conv_batched_multimasks (segment_reduce).

Self-contained: accepts FULL inputs, shards the B*N supervoxel areas across
8 NeuronCores (fully data-parallel), runs one SPMD Bass/Tile kernel, and
gathers the full [B, N] output.

v3 math (numpy-validated against the exact reference, tolerance 2e-2):
 - hdr(x) = dr(dr(x)) == 0.5 + 0.5*erf(7.0898*(x-0.5)) to ~4e-4 abs.
 - The per-channel composite dr(hdr(affine(hdr(m), hdr(mid)))) collapses to
   d = sigmoid(U * wh) with U = erf(7.0898*(m-0.5)),
   wh = 16.545*erf(7.0898*(mid-0.5)) (logistic~probit, k=1.65 tuned
   end-to-end). Final rel-L2 ~4e-3 fp32 / ~6e-3 fp16.
 - The and-tree pair products and their diff_round (Sin) plus the whole
   erosion/stats chain are EXACT (approximating them fails tolerance).

Scheduling: Erf/Sigmoid and Sin live in different ACT function-table sets
(1283ns reload each). Chunks are processed in super-chunks of SB: all
Erf/Sigmoid work for SB chunks, then all Sin work, so tables swap twice
per super-chunk instead of twice per chunk.

fp16 on-device (DVE TensorTensor hits 2x mode for packed 2-byte operands);
reductions and the small per-area tail are fp32. d is written
channel-planar via the ACT output access pattern so both and-tree products
form one packed TensorTensor; r is pair-planar so mc = r0*r1 is packed.
"""

import numpy as np

import concourse.bacc as bacc
import concourse.mybir as mybir
from concourse.tile import TileContext

B, N, W, H = 8, 8192, 8, 8
PX = W * H
CH = 4
NCORES = 8
A_TOT = B * N
A_CORE = A_TOT // NCORES
P = 128

_BUILDER_SRC = r'''
F16 = mybir.dt.float16
F32 = mybir.dt.float32
AX = mybir.AxisListType
OP = mybir.AluOpType
ACTF = mybir.ActivationFunctionType

PI = float(np.pi)
TWO_PI = 2.0 * PI
INV_2PI = 1.0 / TWO_PI
EPS = 1e-8

A_U = 7.0898154036220635          # erf fit of harder_diff_round
K_SIG = 1.65                       # logistic-vs-probit fit (end-to-end tuned)
A_V = 14.179630807244127
C_WH = K_SIG * float(np.sqrt(2.0)) * A_V * 0.5
WH_CLIP = 17.0                     # > C_WH/2*2=16.55 actual range; guards HW table only


def _build(g=8, sb=2, bufs_a=3, bufs_b=4, r_eng="dve"):
    G = g
    SB = sb                        # chunks per super-chunk (table-swap batch)
    CHUNKS = A_CORE // (P * G)
    FD = G * PX * CH               # mask elements per partition per chunk
    FE = G * PX                    # img/edge elements per partition per chunk
    FQ = G * PX * 2                # pair-stage elements

    nc = bacc.Bacc("TRN2", target_bir_lowering=False, debug=False,
                   num_devices=NCORES)
    img_d = nc.dram_tensor("img", [A_CORE, PX], F16, kind="ExternalInput")
    mask_d = nc.dram_tensor("mask", [A_CORE, PX * CH], F16, kind="ExternalInput")
    mid_d = nc.dram_tensor("mid", [A_CORE, CH], F32, kind="ExternalInput")
    edge_d = nc.dram_tensor("edge", [A_CORE, PX], F16, kind="ExternalInput")
    out_d = nc.dram_tensor("out", [A_CORE], F32, kind="ExternalOutput")

    img_v = img_d.ap().rearrange("(c p g) x -> c p (g x)", p=P, g=G)
    mask_v = mask_d.ap().rearrange("(c p g) x -> c p (g x)", p=P, g=G)
    mid_v = mid_d.ap().rearrange("(c p g) x -> c p (g x)", p=P, g=G)
    edge_v = edge_d.ap().rearrange("(c p g) x -> c p (g x)", p=P, g=G)
    out_v = out_d.ap().rearrange("(c p g) -> c p g", p=P, g=G)

    with TileContext(nc) as tc:
        with tc.tile_pool(name="cpool", bufs=1) as cpool, \
             tc.tile_pool(name="pool", bufs=bufs_a) as pool:
            bias_n = cpool.tile([P, 1], F32, name="bias_n")    # -pi
            bias_u = cpool.tile([P, 1], F32, name="bias_u")    # -A_U/2
            bias_1 = cpool.tile([P, 1], F32, name="bias_1")    # +1
            nc.vector.memset(bias_n[:, :], -PI)
            nc.vector.memset(bias_u[:, :], -0.5 * A_U)
            nc.vector.memset(bias_1[:, :], 1.0)
            BNp = bias_n[:, :]
            BUp = bias_u[:, :]
            B1p = bias_1[:, :]

            def phase_a1(c):
                """DMA + Erf stage for one chunk; returns live tiles."""
                mask_t = pool.tile([P, FD], F16, name="mask_t", bufs=SB + 1)
                img_t = pool.tile([P, FE], F16, name="img_t", bufs=SB + 2)
                edge_t = pool.tile([P, FE], F16, name="edge_t", bufs=SB + 2)
                mid_t = pool.tile([P, G * CH], F32, name="mid_t", bufs=SB + 1)
                nc.sync.dma_start(mask_t[:, :], mask_v[c])
                nc.sync.dma_start(img_t[:, :], img_v[c])
                nc.sync.dma_start(edge_t[:, :], edge_v[c])
                nc.sync.dma_start(mid_t[:, :], mid_v[c])

                # wh = clip(C_WH * erf(A_U*(mid-0.5)), +-WH_CLIP)
                em = pool.tile([P, G * CH], F32, name="em", bufs=SB + 1)
                wh = pool.tile([P, G * CH], F16, name="wh", bufs=SB + 1)
                nc.scalar.activation(em[:, :], mid_t[:, :], ACTF.Erf,
                                     scale=A_U, bias=BUp)
                # phase-A DVE ops jump the queue: they feed the sigmoid and
                # must not straggle behind the previous super-chunk's B work
                # (a late sigmoid forces extra ACT table swaps).
                with tc.high_priority():
                    nc.vector.tensor_scalar(em[:, :], em[:, :], C_WH, WH_CLIP,
                                            op0=OP.mult, op1=OP.min)
                    nc.vector.tensor_scalar(wh[:, :], em[:, :], -WH_CLIP, None,
                                            op0=OP.max)

                # U = erf(A_U*(m-0.5))
                U = pool.tile([P, FD], F16, name="U", bufs=SB + 1)
                nc.scalar.activation(U[:, :], mask_t[:, :], ACTF.Erf,
                                     scale=A_U, bias=BUp)
                y = pool.tile([P, FD], F16, name="y", bufs=SB + 1)
                y4 = y[:, :].rearrange("p (g x c) -> p g x c", g=G, c=CH)
                U4 = U[:, :].rearrange("p (g x c) -> p g x c", g=G, c=CH)
                wh_g = wh[:, :].rearrange("p (g c) -> p g c", g=G)
                wh_bc = wh_g.unsqueeze(2).broadcast_to([P, G, PX, CH])
                with tc.high_priority():
                    nc.vector.tensor_tensor(y4, U4, wh_bc, op=OP.mult)
                return y, img_t, edge_t

            def phase_a2(y, *rest):
                """Sigmoid stage: d written channel-planar."""
                y4 = y[:, :].rearrange("p (g x c) -> p g x c", g=G, c=CH)
                d = pool.tile([P, FD], F16, name="d", bufs=SB + 2)
                d_pl = d[:, :].rearrange("p (ch gx) -> p ch gx", ch=CH)
                d_out = d_pl.rearrange("p ch (g x) -> p g x ch", g=G)
                nc.scalar.activation(d_out, y4, ACTF.Sigmoid)
                return (d,) + rest

            def phase_b(c, d, img_t, edge_t):
                """Sin stage + erosion/stats for one chunk."""
                # and-tree: q_j = d_{2j}*d_{2j+1}, both pairs in one packed TT
                q = pool.tile([P, FQ], F16, name="q", bufs=bufs_b)
                d4 = d[:, :].rearrange("p (j u e) -> p j u e", j=2, u=2)
                dA = d4[:, :, 0, :]          # planes {0, 2}
                dB = d4[:, :, 1, :]          # planes {1, 3}
                q_v = q[:, :].rearrange("p (j e) -> p j e", j=2)
                nc.vector.tensor_tensor(q_v, dA, dB, op=OP.mult)
                # r = dr(q) = q - sin(2pi q)/2pi, exact
                sq = pool.tile([P, FQ], F16, name="sq", bufs=bufs_b)
                r = pool.tile([P, FQ], F16, name="r", bufs=bufs_b)
                nc.scalar.activation(sq[:, :], q[:, :], ACTF.Sin,
                                     scale=TWO_PI, bias=BNp)
                # STT is DVE-only on real HW (walrus rejects it on Pool)
                nc.vector.scalar_tensor_tensor(r[:, :], sq[:, :], INV_2PI,
                                               q[:, :], op0=OP.mult,
                                               op1=OP.add)

                # mc = r0*r1 into padded [P, G, 10, 8] tile (rows 0,9 zero)
                mcp = pool.tile([P, G * 80], F16, name="mcp", bufs=bufs_b)
                mcp_v = mcp[:, :].rearrange("p (g w h) -> p g w h", g=G, w=10)
                mcp3 = mcp[:, :].rearrange("p (g e) -> p g e", g=G)
                nc.gpsimd.memset(mcp_v[:, :, 0, :], 0.0)
                nc.gpsimd.memset(mcp_v[:, :, 9, :], 0.0)
                mcc = mcp3[:, :, 8:72]
                r0 = r[:, 0:G * PX].rearrange("p (g e) -> p g e", g=G)
                r1 = r[:, G * PX:].rearrange("p (g e) -> p g e", g=G)
                nc.vector.tensor_tensor(mcc, r0, r1, op=OP.mult)

                # erosion: sum((1 - (ab+af-2*ab*af)*mc)^2 * mc * edge)
                ab = mcp3[:, :, 16:80]
                af = mcp3[:, :, 0:64]
                e1 = pool.tile([P, FE], F16, name="e1", bufs=bufs_b)
                e2 = pool.tile([P, FE], F16, name="e2", bufs=bufs_b)
                hh = pool.tile([P, FE], F16, name="hh", bufs=bufs_b)

                def vE(t):
                    return t[:, :].rearrange("p (g x) -> p g x", g=G)

                nc.gpsimd.tensor_tensor(vE(e1), ab, af, op=OP.mult)
                nc.gpsimd.tensor_tensor(vE(e2), ab, af, op=OP.add)
                nc.vector.scalar_tensor_tensor(hh[:, :], e1[:, :], -2.0,
                                               e2[:, :], op0=OP.mult,
                                               op1=OP.add)
                s = pool.tile([P, FE], F16, name="s", bufs=bufs_b)
                nc.vector.tensor_tensor(vE(s), vE(hh), mcc, op=OP.mult)
                sqe = pool.tile([P, FE], F16, name="sqe", bufs=bufs_b)
                nc.scalar.activation(sqe[:, :], s[:, :], ACTF.Square,
                                     scale=-1.0, bias=B1p)
                me = pool.tile([P, FE], F16, name="me", bufs=bufs_b)
                nc.gpsimd.tensor_tensor(vE(me), mcc, vE(edge_t), op=OP.mult)
                pe = pool.tile([P, FE], F16, name="pe", bufs=bufs_b)
                nc.gpsimd.tensor_tensor(pe[:, :], sqe[:, :], me[:, :],
                                        op=OP.mult)
                sw2 = pool.tile([P, G], F32, name="sw2", bufs=bufs_b)
                nc.vector.reduce_sum(sw2[:, :], vE(pe), axis=AX.X)

                # stats
                z = pool.tile([P, FE], F16, name="z", bufs=bufs_b)
                nc.gpsimd.tensor_tensor(vE(z), mcc, vE(img_t), op=OP.mult)
                sz = pool.tile([P, G], F32, name="sz", bufs=bufs_b)
                smc = pool.tile([P, G], F32, name="smc", bufs=bufs_b)
                nc.vector.reduce_sum(sz[:, :], vE(z), axis=AX.X)
                nc.vector.reduce_sum(smc[:, :], mcc, axis=AX.X)
                rec = pool.tile([P, G], F32, name="rec", bufs=bufs_b)
                nc.vector.tensor_scalar(rec[:, :], smc[:, :], EPS, None,
                                        op0=OP.add)
                nc.vector.reciprocal(rec[:, :], rec[:, :])
                meann = pool.tile([P, G], F32, name="meann", bufs=bufs_b)
                nc.vector.tensor_tensor(meann[:, :], sz[:, :], rec[:, :],
                                        op=OP.mult)
                meann_bc = meann[:, :].unsqueeze(2).broadcast_to([P, G, PX])
                y0 = pool.tile([P, FE], F16, name="y0", bufs=bufs_b)
                nc.gpsimd.tensor_tensor(vE(y0), vE(z), meann_bc,
                                        op=OP.subtract)
                y1 = pool.tile([P, FE], F16, name="y1", bufs=bufs_b)
                nc.vector.tensor_tensor(vE(y1), vE(y0), mcc, op=OP.mult)
                y2 = pool.tile([P, FE], F16, name="y2", bufs=bufs_b)
                nc.vector.tensor_tensor(y2[:, :], y1[:, :], y1[:, :],
                                        op=OP.mult)
                sy2 = pool.tile([P, G], F32, name="sy2", bufs=bufs_b)
                nc.vector.reduce_sum(sy2[:, :], vE(y2), axis=AX.X)
                varr = pool.tile([P, G], F32, name="varr", bufs=bufs_b)
                nc.vector.tensor_tensor(varr[:, :], sy2[:, :], rec[:, :],
                                        op=OP.mult)
                outv = pool.tile([P, G], F32, name="outv", bufs=bufs_b)
                nc.vector.scalar_tensor_tensor(outv[:, :], varr[:, :],
                                               1000.0 / PX, sw2[:, :],
                                               op0=OP.mult, op1=OP.mult)
                nc.sync.dma_start(out_v[c], outv[:, :])

            for sc in range(0, CHUNKS, SB):
                rng = range(sc, min(sc + SB, CHUNKS))
                live = [phase_a1(c) for c in rng]
                live = [phase_a2(*t) for t in live]
                for i, c in enumerate(rng):
                    phase_b(c, *live[i])
    nc.compile()
    return nc
'''

_ns = {
    "np": np,
    "bacc": bacc,
    "mybir": mybir,
    "TileContext": TileContext,
    "B": B, "N": N, "W": W, "H": H, "PX": PX, "CH": CH,
    "NCORES": NCORES, "A_TOT": A_TOT, "A_CORE": A_CORE, "P": P,
}
exec(compile(_BUILDER_SRC, "<bass_kernel>", "exec"), _ns)
_build = _ns["_build"]

_NC = None


def _get_nc():
    global _NC
    if _NC is None:
        _NC = _build()
    return _NC


def kernel(resized_image, mask_combined, initial_masks, edge_map, mask_index=0):
    from concourse.bass_utils import run_bass_kernel_spmd
    img = np.ascontiguousarray(resized_image, dtype=np.float16).reshape(A_TOT, PX)
    mask = np.ascontiguousarray(mask_combined, dtype=np.float16).reshape(A_TOT, PX * CH)
    mid = np.ascontiguousarray(initial_masks, dtype=np.float32).reshape(A_TOT, CH)
    edge = np.ascontiguousarray(edge_map, dtype=np.float16).reshape(A_TOT, PX)

    nc = _get_nc()
    in_maps = []
    for k in range(NCORES):
        sl = slice(k * A_CORE, (k + 1) * A_CORE)
        in_maps.append({
            "img": np.ascontiguousarray(img[sl]),
            "mask": np.ascontiguousarray(mask[sl]),
            "mid": np.ascontiguousarray(mid[sl]),
            "edge": np.ascontiguousarray(edge[sl]),
        })
    res = run_bass_kernel_spmd(nc, in_maps, core_ids=list(range(NCORES)))
    out = np.concatenate([res.results[k]["out"].reshape(-1)
                          for k in range(NCORES)])
    return out.reshape(B, N).astype(np.float32)


# revision 23
# speedup vs baseline: 2.1443x; 1.0110x over previous
"""Trainium2 Bass kernel for nn_De_conv_batched_multimasks (segment_reduce).

Self-contained: accepts FULL inputs, shards the B*N supervoxel areas across
8 NeuronCores (fully data-parallel), runs one SPMD Bass/Tile kernel, and
gathers the full [B, N] output.

v3 math (numpy-validated against the exact reference, tolerance 2e-2):
 - hdr(x) = dr(dr(x)) == 0.5 + 0.5*erf(7.0898*(x-0.5)) to ~4e-4 abs.
 - The per-channel composite dr(hdr(affine(hdr(m), hdr(mid)))) collapses to
   d = sigmoid(U * wh) with U = erf(7.0898*(m-0.5)),
   wh = 16.545*erf(7.0898*(mid-0.5)) (logistic~probit, k=1.65 tuned
   end-to-end). Final rel-L2 ~4e-3 fp32 / ~6e-3 fp16.
 - The and-tree pair products and their diff_round (Sin) plus the whole
   erosion/stats chain are EXACT (approximating them fails tolerance).

Scheduling: Erf/Sigmoid and Sin live in different ACT function-table sets
(1283ns reload each). Chunks are processed in super-chunks of SB: all
Erf/Sigmoid work for SB chunks, then all Sin work, so tables swap twice
per super-chunk instead of twice per chunk.

fp16 on-device (DVE TensorTensor hits 2x mode for packed 2-byte operands);
reductions and the small per-area tail are fp32. d is written
channel-planar via the ACT output access pattern so both and-tree products
form one packed TensorTensor; r is pair-planar so mc = r0*r1 is packed.
"""

import numpy as np

import concourse.bacc as bacc
import concourse.mybir as mybir
from concourse.tile import TileContext

B, N, W, H = 8, 8192, 8, 8
PX = W * H
CH = 4
NCORES = 8
A_TOT = B * N
A_CORE = A_TOT // NCORES
P = 128

_BUILDER_SRC = r'''
F16 = mybir.dt.float16
F32 = mybir.dt.float32
AX = mybir.AxisListType
OP = mybir.AluOpType
ACTF = mybir.ActivationFunctionType

PI = float(np.pi)
TWO_PI = 2.0 * PI
INV_2PI = 1.0 / TWO_PI
EPS = 1e-8

A_U = 7.0898154036220635          # erf fit of harder_diff_round
K_SIG = 1.65                       # logistic-vs-probit fit (end-to-end tuned)
A_V = 14.179630807244127
C_WH = K_SIG * float(np.sqrt(2.0)) * A_V * 0.5
WH_CLIP = 17.0                     # > C_WH/2*2=16.55 actual range; guards HW table only


def _build(g=8, sb=2, bufs_a=3, bufs_b=4, r_eng="dve"):
    G = g
    SB = sb                        # chunks per super-chunk (table-swap batch)
    CHUNKS = A_CORE // (P * G)
    FD = G * PX * CH               # mask elements per partition per chunk
    FE = G * PX                    # img/edge elements per partition per chunk
    FQ = G * PX * 2                # pair-stage elements

    nc = bacc.Bacc("TRN2", target_bir_lowering=False, debug=False,
                   num_devices=NCORES)
    # "mask" rows carry the 256 mask values + the 4 mid values appended, so
    # one Erf activation covers both (host concatenates them).
    MR = PX * CH + CH
    img_d = nc.dram_tensor("img", [A_CORE, PX], F16, kind="ExternalInput")
    mask_d = nc.dram_tensor("mask", [A_CORE, MR], F16, kind="ExternalInput")
    edge_d = nc.dram_tensor("edge", [A_CORE, PX], F16, kind="ExternalInput")
    out_d = nc.dram_tensor("out", [A_CORE], F32, kind="ExternalOutput")

    img_v = img_d.ap().rearrange("(c p g) x -> c p (g x)", p=P, g=G)
    mask_v = mask_d.ap().rearrange("(c p g) x -> c p (g x)", p=P, g=G)
    edge_v = edge_d.ap().rearrange("(c p g) x -> c p (g x)", p=P, g=G)
    out_v = out_d.ap().rearrange("(c p g) -> c p g", p=P, g=G)

    with TileContext(nc) as tc:
        with tc.tile_pool(name="cpool", bufs=1) as cpool, \
             tc.tile_pool(name="pool", bufs=bufs_a) as pool:
            bias_n = cpool.tile([P, 1], F32, name="bias_n")    # -pi
            bias_u = cpool.tile([P, 1], F32, name="bias_u")    # -A_U/2
            bias_1 = cpool.tile([P, 1], F32, name="bias_1")    # +1
            nc.vector.memset(bias_n[:, :], -PI)
            nc.vector.memset(bias_u[:, :], -0.5 * A_U)
            nc.vector.memset(bias_1[:, :], 1.0)
            BNp = bias_n[:, :]
            BUp = bias_u[:, :]
            B1p = bias_1[:, :]

            def phase_a1(c):
                """DMA + Erf stage for one chunk; returns live tiles."""
                mask_t = pool.tile([P, G * MR], F16, name="mask_t", bufs=SB + 1)
                img_t = pool.tile([P, FE], F16, name="img_t", bufs=SB + 2)
                edge_t = pool.tile([P, FE], F16, name="edge_t", bufs=SB + 2)
                nc.sync.dma_start(mask_t[:, :], mask_v[c])
                nc.sync.dma_start(img_t[:, :], img_v[c])
                nc.sync.dma_start(edge_t[:, :], edge_v[c])

                # one Erf covers the 256 mask values and 4 appended mid values
                U = pool.tile([P, G * MR], F16, name="U", bufs=SB + 1)
                nc.scalar.activation(U[:, :], mask_t[:, :], ACTF.Erf,
                                     scale=A_U, bias=BUp)
                Ug = U[:, :].rearrange("p (g m) -> p g m", g=G)
                U4 = Ug[:, :, 0:PX * CH].rearrange("p g (x c) -> p g x c",
                                                   c=CH)
                Umid = Ug[:, :, PX * CH:]
                # wh = clip(C_WH * erf(A_U*(mid-0.5)), +-WH_CLIP)
                wh = pool.tile([P, G * CH], F16, name="wh", bufs=SB + 1)
                wh_g = wh[:, :].rearrange("p (g c) -> p g c", g=G)
                # phase-A DVE ops jump the queue: they feed the sigmoid and
                # must not straggle behind the previous super-chunk's B work
                # (a late sigmoid forces extra ACT table swaps).
                with tc.high_priority():
                    nc.vector.tensor_scalar(wh_g, Umid, C_WH, WH_CLIP,
                                            op0=OP.mult, op1=OP.min)
                    nc.vector.tensor_scalar(wh[:, :], wh[:, :], -WH_CLIP, None,
                                            op0=OP.max)
                y = pool.tile([P, FD], F16, name="y", bufs=SB + 1)
                y4 = y[:, :].rearrange("p (g x c) -> p g x c", g=G, c=CH)
                wh_bc = wh_g.unsqueeze(2).broadcast_to([P, G, PX, CH])
                with tc.high_priority():
                    nc.vector.tensor_tensor(y4, U4, wh_bc, op=OP.mult)
                return y, img_t, edge_t

            def phase_a2(y, *rest):
                """Sigmoid stage: d written channel-planar."""
                y4 = y[:, :].rearrange("p (g x c) -> p g x c", g=G, c=CH)
                d = pool.tile([P, FD], F16, name="d", bufs=SB + 2)
                d_pl = d[:, :].rearrange("p (ch gx) -> p ch gx", ch=CH)
                d_out = d_pl.rearrange("p ch (g x) -> p g x ch", g=G)
                nc.scalar.activation(d_out, y4, ACTF.Sigmoid)
                return (d,) + rest

            def phase_b(c, d, img_t, edge_t):
                """Sin stage + erosion/stats for one chunk."""
                # and-tree: q_j = d_{2j}*d_{2j+1}, both pairs in one packed TT
                q = pool.tile([P, FQ], F16, name="q", bufs=bufs_b)
                d4 = d[:, :].rearrange("p (j u e) -> p j u e", j=2, u=2)
                dA = d4[:, :, 0, :]          # planes {0, 2}
                dB = d4[:, :, 1, :]          # planes {1, 3}
                q_v = q[:, :].rearrange("p (j e) -> p j e", j=2)
                nc.vector.tensor_tensor(q_v, dA, dB, op=OP.mult)
                # r = dr(q) = q - sin(2pi q)/2pi, exact
                sq = pool.tile([P, FQ], F16, name="sq", bufs=bufs_b)
                r = pool.tile([P, FQ], F16, name="r", bufs=bufs_b)
                nc.scalar.activation(sq[:, :], q[:, :], ACTF.Sin,
                                     scale=TWO_PI, bias=BNp)
                # STT is DVE-only on real HW (walrus rejects it on Pool)
                nc.vector.scalar_tensor_tensor(r[:, :], sq[:, :], INV_2PI,
                                               q[:, :], op0=OP.mult,
                                               op1=OP.add)

                # mc = r0*r1 into padded [P, G, 10, 8] tile (rows 0,9 zero)
                mcp = pool.tile([P, G * 80], F16, name="mcp", bufs=bufs_b)
                mcp_v = mcp[:, :].rearrange("p (g w h) -> p g w h", g=G, w=10)
                mcp3 = mcp[:, :].rearrange("p (g e) -> p g e", g=G)
                nc.gpsimd.memset(mcp_v[:, :, 0, :], 0.0)
                nc.gpsimd.memset(mcp_v[:, :, 9, :], 0.0)
                mcc = mcp3[:, :, 8:72]
                r0 = r[:, 0:G * PX].rearrange("p (g e) -> p g e", g=G)
                r1 = r[:, G * PX:].rearrange("p (g e) -> p g e", g=G)
                nc.vector.tensor_tensor(mcc, r0, r1, op=OP.mult)

                # erosion: sum((1 - (ab+af-2*ab*af)*mc)^2 * mc * edge)
                ab = mcp3[:, :, 16:80]
                af = mcp3[:, :, 0:64]
                e1 = pool.tile([P, FE], F16, name="e1", bufs=bufs_b)
                e2 = pool.tile([P, FE], F16, name="e2", bufs=bufs_b)
                hh = pool.tile([P, FE], F16, name="hh", bufs=bufs_b)

                def vE(t):
                    return t[:, :].rearrange("p (g x) -> p g x", g=G)

                nc.gpsimd.tensor_tensor(vE(e1), ab, af, op=OP.mult)
                nc.gpsimd.tensor_tensor(vE(e2), ab, af, op=OP.add)
                nc.vector.scalar_tensor_tensor(hh[:, :], e1[:, :], -2.0,
                                               e2[:, :], op0=OP.mult,
                                               op1=OP.add)
                s = pool.tile([P, FE], F16, name="s", bufs=bufs_b)
                nc.vector.tensor_tensor(vE(s), vE(hh), mcc, op=OP.mult)
                sqe = pool.tile([P, FE], F16, name="sqe", bufs=bufs_b)
                nc.scalar.activation(sqe[:, :], s[:, :], ACTF.Square,
                                     scale=-1.0, bias=B1p)
                me = pool.tile([P, FE], F16, name="me", bufs=bufs_b)
                nc.gpsimd.tensor_tensor(vE(me), mcc, vE(edge_t), op=OP.mult)
                pe = pool.tile([P, FE], F16, name="pe", bufs=bufs_b)
                nc.gpsimd.tensor_tensor(pe[:, :], sqe[:, :], me[:, :],
                                        op=OP.mult)
                sw2 = pool.tile([P, G], F32, name="sw2", bufs=bufs_b)
                nc.vector.reduce_sum(sw2[:, :], vE(pe), axis=AX.X)

                # stats
                z = pool.tile([P, FE], F16, name="z", bufs=bufs_b)
                nc.vector.tensor_tensor(vE(z), mcc, vE(img_t), op=OP.mult)
                sz = pool.tile([P, G], F32, name="sz", bufs=bufs_b)
                smc = pool.tile([P, G], F32, name="smc", bufs=bufs_b)
                nc.vector.reduce_sum(sz[:, :], vE(z), axis=AX.X)
                nc.vector.reduce_sum(smc[:, :], mcc, axis=AX.X)
                rec = pool.tile([P, G], F32, name="rec", bufs=bufs_b)
                nc.vector.tensor_scalar(rec[:, :], smc[:, :], EPS, None,
                                        op0=OP.add)
                nc.vector.reciprocal(rec[:, :], rec[:, :])
                meann = pool.tile([P, G], F32, name="meann", bufs=bufs_b)
                nc.vector.tensor_tensor(meann[:, :], sz[:, :], rec[:, :],
                                        op=OP.mult)
                meann_bc = meann[:, :].unsqueeze(2).broadcast_to([P, G, PX])
                y0 = pool.tile([P, FE], F16, name="y0", bufs=bufs_b)
                nc.vector.scalar_tensor_tensor(vE(y0), meann_bc, -1.0, vE(z),
                                               op0=OP.mult, op1=OP.add)
                y1 = pool.tile([P, FE], F16, name="y1", bufs=bufs_b)
                nc.vector.tensor_tensor(vE(y1), vE(y0), mcc, op=OP.mult)
                y2 = pool.tile([P, FE], F16, name="y2", bufs=bufs_b)
                nc.vector.tensor_tensor(y2[:, :], y1[:, :], y1[:, :],
                                        op=OP.mult)
                sy2 = pool.tile([P, G], F32, name="sy2", bufs=bufs_b)
                nc.vector.reduce_sum(sy2[:, :], vE(y2), axis=AX.X)
                varr = pool.tile([P, G], F32, name="varr", bufs=bufs_b)
                nc.vector.tensor_tensor(varr[:, :], sy2[:, :], rec[:, :],
                                        op=OP.mult)
                outv = pool.tile([P, G], F32, name="outv", bufs=bufs_b)
                nc.vector.scalar_tensor_tensor(outv[:, :], varr[:, :],
                                               1000.0 / PX, sw2[:, :],
                                               op0=OP.mult, op1=OP.mult)
                nc.sync.dma_start(out_v[c], outv[:, :])

            for sc in range(0, CHUNKS, SB):
                rng = range(sc, min(sc + SB, CHUNKS))
                live = [phase_a1(c) for c in rng]
                live = [phase_a2(*t) for t in live]
                for i, c in enumerate(rng):
                    phase_b(c, *live[i])
    nc.compile()
    return nc
'''

_ns = {
    "np": np,
    "bacc": bacc,
    "mybir": mybir,
    "TileContext": TileContext,
    "B": B, "N": N, "W": W, "H": H, "PX": PX, "CH": CH,
    "NCORES": NCORES, "A_TOT": A_TOT, "A_CORE": A_CORE, "P": P,
}
exec(compile(_BUILDER_SRC, "<bass_kernel>", "exec"), _ns)
_build = _ns["_build"]

_NC = None


def _get_nc():
    global _NC
    if _NC is None:
        _NC = _build()
    return _NC


def kernel(resized_image, mask_combined, initial_masks, edge_map, mask_index=0):
    from concourse.bass_utils import run_bass_kernel_spmd
    img = np.ascontiguousarray(resized_image, dtype=np.float16).reshape(A_TOT, PX)
    mask = np.asarray(mask_combined, dtype=np.float16).reshape(A_TOT, PX * CH)
    mid = np.asarray(initial_masks, dtype=np.float16).reshape(A_TOT, CH)
    maskmid = np.ascontiguousarray(np.concatenate([mask, mid], axis=1))
    edge = np.ascontiguousarray(edge_map, dtype=np.float16).reshape(A_TOT, PX)

    nc = _get_nc()
    in_maps = []
    for k in range(NCORES):
        sl = slice(k * A_CORE, (k + 1) * A_CORE)
        in_maps.append({
            "img": np.ascontiguousarray(img[sl]),
            "mask": np.ascontiguousarray(maskmid[sl]),
            "edge": np.ascontiguousarray(edge[sl]),
        })
    res = run_bass_kernel_spmd(nc, in_maps, core_ids=list(range(NCORES)))
    out = np.concatenate([res.results[k]["out"].reshape(-1)
                          for k in range(NCORES)])
    return out.reshape(B, N).astype(np.float32)
